# revision 38
# baseline (speedup 1.0000x reference)
"""Multi-head attention Trainium2 Bass kernel.

Problem: B=2, S=2048, D=1024, H=16, HS=64.
Sharding: tensor-parallel over heads — each of 8 cores computes 2 heads
(128 contiguous output-feature columns) for both batches; host concatenates.

Per-core pipeline (v2 — fully dataflow-overlapped):
  * Projections per batch in bf16 (PSUM fp32): Q^T/K^T feature-major with the
    bias folded into the PSUM->SBUF cast on the DVE (tensor_scalar_add with a
    per-partition bias column — no K=1 bias matmuls); V' token-major with the
    softmax-denominator ones column folded into the weight matrix.
  * Attention in (batch, 512-query) units.  Per k-chunk of 128 tokens, ONE
    [128, 1024] PSUM tile holds both heads' sims side by side; the two sim
    matmuls (K=64 each) target disjoint PE row groups via tile_position and
    become ready simultaneously (single tile release), so the PE streams them
    concurrently (~2x).  ONE exp covers both heads.  O'^T[65, q] += V'^T P^T
    accumulates per head in its own PSUM bank (row 64 = denominator).
  * PSUM budget: 2 banks proj pool + 4 banks sim pool + 2 banks PV pool = 8.
    The dedicated proj pool lets batch-1 projections fill Tensor-engine gaps
    during batch-0 attention (the exp stream on the Scalar engine is the
    critical resource there).
  * Unnormalized O'^T goes straight to DRAM; the host divides and transposes.
"""

import sys

sys.path.insert(0, "/opt/trn_rl_repo")

import ml_dtypes
import numpy as np

import concourse.bass as bass
import concourse.mybir as mybir
import concourse.tile as tile
from concourse import bacc
from concourse import bass_utils

B, S, D = 2, 2048, 1024
H, HS = 16, 64
NCORES = 8
NTOK = B * S                  # 4096
FPC = (H // NCORES) * HS      # 128 output-feature cols per core (2 heads)
TT = 512                      # token tile for projections (== QT)
NTPB = S // TT                # 4 t-tiles per batch
NCH = D // 128                # 8 contraction chunks
QT = 512                      # query width per attention unit
NU = S // QT                  # 4 units per batch
KT = 128                      # k chunk in attention
NKT = S // KT                 # 16
VW = 2 * (HS + 1)             # 130: [V_h0 | 1 | V_h1 | 1] columns

F32 = mybir.dt.float32
BF16 = mybir.dt.bfloat16

_NC_CACHE = {}


def build_nc():
    nc = bacc.Bacc("TRN2", target_bir_lowering=False, debug=False, num_devices=NCORES)
    xt = nc.dram_tensor("xt", [D, NTOK], BF16, kind="ExternalInput").ap()
    # Weights arrive pre-laid-out in their SBUF shape and already in bf16
    # (the host does the cheap transpose+cast) so each loads with ONE
    # contiguous DMA on the fast sync/HWDGE queue and needs no on-chip cast.
    wq = nc.dram_tensor("wq", [128, NCH * FPC], BF16, kind="ExternalInput").ap()
    wk = nc.dram_tensor("wk", [128, NCH * FPC], BF16, kind="ExternalInput").ap()
    wvp = nc.dram_tensor("wvp", [128, NCH * VW], BF16, kind="ExternalInput").ap()
    bqc = nc.dram_tensor("bqc", [FPC, 1], F32, kind="ExternalInput").ap()
    bkc = nc.dram_tensor("bkc", [FPC, 1], F32, kind="ExternalInput").ap()
    out = nc.dram_tensor("out", [2 * (HS + 1), NTOK], F32, kind="ExternalOutput").ap()

    with tile.TileContext(nc) as tc:
        with (
            tc.tile_pool(name="persist", bufs=1) as pp,
            tc.tile_pool(name="work", bufs=2) as wkp,
            tc.tile_pool(name="psProj", bufs=2, space="PSUM") as psProj,
            tc.tile_pool(name="psSim", bufs=2, space="PSUM") as psSim,
            tc.tile_pool(name="psPV", bufs=2, space="PSUM") as psPV,
        ):
            # ---------------- init: weights, biases, X^T ---------------------
            wq_b = pp.tile([128, NCH * FPC], BF16)
            wk_b = pp.tile([128, NCH * FPC], BF16)
            wv_b = pp.tile([128, NCH * VW], BF16)
            bq_sb = pp.tile([128, 1], F32)
            bk_sb = pp.tile([128, 1], F32)

            # X^T per (batch, chunk): fine-grained tiles so projection
            # dependencies resolve per DMA, batch 0 first.
            xtc = [
                [pp.tile([128, S], BF16, name=f"xt_{b}_{c}") for c in range(NCH)]
                for b in range(B)
            ]

            # Small bias DMAs on the SWDGE (gpsimd) queue.  Each sync/HWDGE
            # dma_start costs ~650ns of ISSUE time regardless of size, so the
            # head-critical transfers use the fewest possible DMAs: one per
            # weight, one per (batch, chunk) for X^T — ordered wk, wq, batch-0
            # X^T (gates the first exp), wv, batch-1 X^T.
            nc.gpsimd.dma_start(bk_sb[:], bkc[:, :])
            nc.gpsimd.dma_start(bq_sb[:], bqc[:, :])
            nc.sync.dma_start(wk_b[:], wk[:, :])
            nc.sync.dma_start(wq_b[:], wq[:, :])
            nc.sync.dma_start(wv_b[:], wvp[:, :])
            for c in range(NCH):
                nc.sync.dma_start(
                    xtc[0][c][:, 0:2*TT], xt[c * 128 : (c + 1) * 128, 0:2*TT]
                )
            for c in range(NCH):
                nc.sync.dma_start(
                    xtc[0][c][:, 2*TT:S], xt[c * 128 : (c + 1) * 128, 2*TT:S]
                )
            for c in range(NCH):
                nc.sync.dma_start(xtc[1][c][:], xt[c * 128 : (c + 1) * 128, S : 2 * S])

            # ---------------- persistent activations ------------------------
            qt_sb = [
                [pp.tile([128, QT], BF16, name=f"qt_{b}_{u}") for u in range(NU)]
                for b in range(B)
            ]
            kt_sb = [
                [pp.tile([128, TT], BF16, name=f"kt_{b}_{t}") for t in range(NTPB)]
                for b in range(B)
            ]
            vp_sb = [
                [pp.tile([128, VW], BF16, name=f"vp_{b}_{j}") for j in range(S // 128)]
                for b in range(B)
            ]

            def proj_tile(b, t, w_b, bias_sb, dst):
                # Generator: two ~4-matmul granules, so deferred projections
                # trace in slack-sized pieces between attention iterations.
                tsl = slice(t * TT, (t + 1) * TT)
                ps = psProj.tile(
                    [128, TT], F32, name=f"pj_{b}_{t}_{dst.tensor.name}", tag="proj"
                )
                for c in range(NCH):
                    nc.tensor.matmul(
                        ps[:], w_b[:, c * FPC : (c + 1) * FPC], xtc[b][c][:, tsl],
                        start=(c == 0), stop=(c == NCH - 1),
                    )
                    if c in (1, 3, 5):
                        yield
                # PSUM->SBUF cast with the bias folded in (per-partition
                # scalar column) — runs on the DVE.
                nc.vector.tensor_scalar_add(dst[:], ps[:], bias_sb[:])
                yield

            def proj_vtile(b, j):
                # No bias matmul: softmax rows sum to 1, so bV is added on the
                # host; the denominator ones-columns are memset after the copy
                # (the wvp columns 64/129 are zero, so the PSUM there is 0).
                psv = psProj.tile([128, VW], F32, name=f"pv_{b}_{j}", tag="proj")
                for c in range(NCH):
                    nc.tensor.matmul(
                        psv[:], xtc[b][c][:, j * 128 : (j + 1) * 128],
                        wv_b[:, c * VW : (c + 1) * VW],
                        start=(c == 0), stop=(c == NCH - 1),
                    )
                    if c in (1, 3, 5):
                        yield
                nc.vector.tensor_copy(vp_sb[b][j][:], psv[:])
                nc.vector.memset(vp_sb[b][j][:, HS : HS + 1], 1.0)
                nc.vector.memset(vp_sb[b][j][:, VW - 1 : VW], 1.0)
                yield

            def chain(gens):
                for g in gens:
                    yield from g

            class StepQ:
                """Deferred-projection step queue: pull(target) traces steps
                until `target` have been traced (deadline-forced); drain_all
                flushes the remainder."""

                def __init__(self, gen):
                    self.it, self.n, self.done = gen, 0, False

                def pull(self, target):
                    while not self.done and self.n < target:
                        try:
                            next(self.it)
                            self.n += 1
                        except StopIteration:
                            self.done = True

                def drain_all(self):
                    self.pull(1 << 30)

            def proj_prefix(b):
                # Traced directly (not as steps): all of K plus Q of unit 0 —
                # the gate for the batch's first exp.  Q right after K(t0): Q
                # only needs the first X^T half, and the proj pool's 2-slot
                # FIFO would otherwise park it behind K(t2/t3)'s DMA wait.
                order = [(wk_b, bk_sb, kt_sb[b][0]), (wq_b, bq_sb, qt_sb[b][0])] + [
                    (wk_b, bk_sb, kt_sb[b][t]) for t in range(1, NTPB)
                ]
                for i, (w_b, bias_sb, dst) in enumerate(order):
                    t = 0 if i < 2 else i - 1
                    StepQ(proj_tile(b, t, w_b, bias_sb, dst)).drain_all()

            # Output staging: units land in persistent SBUF buffers; one big
            # 8KB-row DMA per (batch, head) at batch end (batch 0's overlaps
            # batch-1 attention, batch 1's is a short tail).
            obuf = [
                [pp.tile([HS + 1, S], F32, name=f"ob_{b}_{h}") for h in range(2)]
                for b in range(B)
            ]

            def attn_unit(b, u, sq=None, need=None):
                # sq/need: deferred-projection step queue and its cumulative
                # trace deadline per kt (vp[kt] must be traced before PV(kt)).
                pvp = [
                    psPV.tile([HS + 1, QT], F32, name=f"pvp_{b}_{u}_{h}", tag="pv")
                    for h in range(2)
                ]
                if sq and need:
                    sq.pull(need(0))
                for kt in range(NKT):
                    sim = psSim.tile([128, 2 * QT], F32, name=f"sim_{b}_{u}_{kt}", tag="sim")
                    # Both heads' sims in one tile: the two K=64 matmuls hit
                    # disjoint PE row groups and disjoint PSUM banks, and
                    # become ready together -> concurrent streaming.
                    for h in range(2):
                        hp = h * HS
                        nc.tensor.matmul(
                            sim[:, h * QT : (h + 1) * QT],
                            kt_sb[b][kt // 4][hp : hp + HS, (kt % 4) * KT : (kt % 4 + 1) * KT],
                            qt_sb[b][u][hp : hp + HS, :],
                            start=True, stop=True,
                            tile_position=(hp, 0),
                        )
                    pt = wkp.tile([128, 2 * QT], BF16, name=f"pt_{b}_{u}_{kt}", tag="pt", bufs=8)
                    nc.scalar.activation(
                        pt[:], sim[:], mybir.ActivationFunctionType.Exp, scale=1.0 / np.sqrt(HS)
                    )
                    for h in range(2):
                        nc.tensor.matmul(
                            pvp[h][:],
                            vp_sb[b][kt][:, h * (HS + 1) : (h + 1) * (HS + 1)],
                            pt[:, h * QT : (h + 1) * QT],
                            start=(kt == 0), stop=(kt == NKT - 1),
                        )
                    if sq and need and kt + 1 < NKT:
                        sq.pull(need(kt + 1))
                if sq:
                    sq.drain_all()
                for h in range(2):
                    nc.vector.tensor_copy(obuf[b][h][:, u * QT : (u + 1) * QT], pvp[h][:])
                if u % 2 == 1:
                    lo, hi = (u - 1) * QT, (u + 1) * QT
                    for h in range(2):
                        nc.sync.dma_start(
                            out[h * (HS + 1) : (h + 1) * (HS + 1), b * S + lo : b * S + hi],
                            obuf[b][h][:, lo:hi],
                        )

            # Emission order = scheduler priority; producers always trace
            # before consumers, but deferred projections trace in slack-sized
            # granules between attention iterations so the exp-paced stream
            # owns the priority and projections fill Tensor-engine gaps.
            def qgen(b, u):
                return proj_tile(b, u, wq_b, bq_sb, qt_sb[b][u])

            def vq_queue(b):
                # V' chunks (4 steps each) with the next unit's Q chain
                # embedded mid-queue so the Q->cast->sim chain never lands on
                # the unit boundary (a late PV is absorbed by the pt buffers;
                # a late sim stalls the exp stream directly).
                gens = [proj_vtile(b, j) for j in range(9)] + [qgen(b, 1)] + [
                    proj_vtile(b, j) for j in range(9, 16)
                ]
                return StepQ(chain(gens))

            def vq_need(kt):
                # Deadline-exact: vp[kt] (4 steps each) traced just before its
                # PV; the embedded Q chain (+4) pulled from kt 8 on, early
                # enough that the proj pool's 2-slot FIFO still executes the
                # Q->cast chain well before the unit boundary.
                return 4 * (kt + 1) + (4 if kt >= 8 else 0)

            proj_prefix(0)
            attn_unit(0, 0, vq_queue(0), need=vq_need)
            attn_unit(
                0, 1,
                StepQ(chain([qgen(0, 2), qgen(0, 3)])),
                need=lambda kt: min(8, (kt + 3) // 2),
            )
            attn_unit(
                0, 2,
                StepQ(chain([proj_tile(1, t, wk_b, bk_sb, kt_sb[1][t]) for t in range(NTPB)])),
                need=lambda kt: kt + 1,
            )
            attn_unit(
                0, 3,
                StepQ(qgen(1, 0)),
                need=lambda kt: min(4, (kt + 1) // 3),
            )
            attn_unit(1, 0, vq_queue(1), need=vq_need)
            attn_unit(
                1, 1,
                StepQ(chain([qgen(1, 2), qgen(1, 3)])),
                need=lambda kt: min(8, (kt + 3) // 2),
            )
            attn_unit(1, 2)
            attn_unit(1, 3)

    nc.compile()
    return nc


def get_nc():
    if "nc" not in _NC_CACHE:
        _NC_CACHE["nc"] = build_nc()
    return _NC_CACHE["nc"]


def make_in_maps(seq_input, WQ, bQ, WK, bK, WV, bV):
    x = np.asarray(seq_input, dtype=np.float32).reshape(NTOK, D)
    xt = np.ascontiguousarray(x.T).astype(ml_dtypes.bfloat16)

    def sbuf_layout(w, width):
        # [D, width] -> [128, NCH*width] bf16: chunk c of D-rows lands at
        # columns [c*width, (c+1)*width) — the exact SBUF image the kernel
        # expects, pre-cast so no on-chip conversion is needed.
        return np.ascontiguousarray(
            w.reshape(NCH, 128, width).transpose(1, 0, 2).reshape(128, NCH * width)
        ).astype(ml_dtypes.bfloat16)

    in_maps = []
    for c in range(NCORES):
        lo, hi = c * FPC, (c + 1) * FPC
        wvp = np.zeros((D, VW), dtype=np.float32)
        wvp[:, 0:HS] = WV[:, lo : lo + HS]
        wvp[:, HS + 1 : 2 * HS + 1] = WV[:, lo + HS : hi]
        in_maps.append(
            {
                "xt": xt,
                "wq": sbuf_layout(np.ascontiguousarray(WQ[:, lo:hi]), FPC),
                "wk": sbuf_layout(np.ascontiguousarray(WK[:, lo:hi]), FPC),
                "wvp": sbuf_layout(wvp, VW),
                "bqc": np.ascontiguousarray(bQ[lo:hi]).reshape(FPC, 1),
                "bkc": np.ascontiguousarray(bK[lo:hi]).reshape(FPC, 1),
            }
        )
    return in_maps


def run(in_maps, trace=False):
    nc = get_nc()
    return bass_utils.run_bass_kernel_spmd(nc, in_maps, core_ids=list(range(NCORES)), trace=trace)


def kernel(seq_input, WQ, bQ, WK, bK, WV, bV):
    in_maps = make_in_maps(
        np.asarray(seq_input, np.float32),
        np.asarray(WQ, np.float32), np.asarray(bQ, np.float32),
        np.asarray(WK, np.float32), np.asarray(bK, np.float32),
        np.asarray(WV, np.float32), np.asarray(bV, np.float32),
    )
    res = run(in_maps)
    bV_np = np.asarray(bV, np.float32)
    parts = []
    for c in range(NCORES):
        o = res.results[c]["out"]  # [130, 4096] feature-major, unnormalized
        for h in range(2):
            lo = c * FPC + h * HS
            num = o[h * (HS + 1) : h * (HS + 1) + HS, :]      # [64, 4096]
            den = o[h * (HS + 1) + HS, :]                     # [4096]
            # softmax rows sum to 1, so the V bias is added after the fact
            parts.append((num / den).T + bV_np[lo : lo + HS])  # [4096, 64]
    full = np.concatenate(parts, axis=1)  # [4096, 1024]
    return full.reshape(B, S, H * HS)


# revision 39
# speedup vs baseline: 1.0063x; 1.0063x over previous
"""Multi-head attention Trainium2 Bass kernel.

Problem: B=2, S=2048, D=1024, H=16, HS=64.
Sharding: tensor-parallel over heads — each of 8 cores computes 2 heads
(128 contiguous output-feature columns) for both batches; host concatenates.

Per-core pipeline (v2 — fully dataflow-overlapped):
  * Projections per batch in bf16 (PSUM fp32): Q^T/K^T feature-major with the
    bias folded into the PSUM->SBUF cast on the DVE (tensor_scalar_add with a
    per-partition bias column — no K=1 bias matmuls); V' token-major with the
    softmax-denominator ones column folded into the weight matrix.
  * Attention in (batch, 512-query) units.  Per k-chunk of 128 tokens, ONE
    [128, 1024] PSUM tile holds both heads' sims side by side; the two sim
    matmuls (K=64 each) target disjoint PE row groups via tile_position and
    become ready simultaneously (single tile release), so the PE streams them
    concurrently (~2x).  ONE exp covers both heads.  O'^T[65, q] += V'^T P^T
    accumulates per head in its own PSUM bank (row 64 = denominator).
  * PSUM budget: 2 banks proj pool + 4 banks sim pool + 2 banks PV pool = 8.
    The dedicated proj pool lets batch-1 projections fill Tensor-engine gaps
    during batch-0 attention (the exp stream on the Scalar engine is the
    critical resource there).
  * Unnormalized O'^T goes straight to DRAM; the host divides and transposes.
"""

import sys

sys.path.insert(0, "/opt/trn_rl_repo")

import ml_dtypes
import numpy as np

import concourse.bass as bass
import concourse.mybir as mybir
import concourse.tile as tile
from concourse import bacc
from concourse import bass_utils

B, S, D = 2, 2048, 1024
H, HS = 16, 64
NCORES = 8
NTOK = B * S                  # 4096
FPC = (H // NCORES) * HS      # 128 output-feature cols per core (2 heads)
TT = 512                      # token tile for projections (== QT)
NTPB = S // TT                # 4 t-tiles per batch
NCH = D // 128                # 8 contraction chunks
QT = 512                      # query width per attention unit
NU = S // QT                  # 4 units per batch
KT = 128                      # k chunk in attention
NKT = S // KT                 # 16
VW = 2 * (HS + 1)             # 130: [V_h0 | 1 | V_h1 | 1] columns

F32 = mybir.dt.float32
BF16 = mybir.dt.bfloat16

_NC_CACHE = {}


def build_nc():
    nc = bacc.Bacc("TRN2", target_bir_lowering=False, debug=False, num_devices=NCORES)
    xt = nc.dram_tensor("xt", [D, NTOK], BF16, kind="ExternalInput").ap()
    # Weights arrive pre-laid-out in their SBUF shape and already in bf16
    # (the host does the cheap transpose+cast) so each loads with ONE
    # contiguous DMA on the fast sync/HWDGE queue and needs no on-chip cast.
    wq = nc.dram_tensor("wq", [128, NCH * FPC], BF16, kind="ExternalInput").ap()
    wk = nc.dram_tensor("wk", [128, NCH * FPC], BF16, kind="ExternalInput").ap()
    wvp = nc.dram_tensor("wvp", [128, NCH * VW], BF16, kind="ExternalInput").ap()
    bqc = nc.dram_tensor("bqc", [FPC, 1], F32, kind="ExternalInput").ap()
    bkc = nc.dram_tensor("bkc", [FPC, 1], F32, kind="ExternalInput").ap()
    out = nc.dram_tensor("out", [2 * (HS + 1), NTOK], F32, kind="ExternalOutput").ap()

    with tile.TileContext(nc) as tc:
        with (
            tc.tile_pool(name="persist", bufs=1) as pp,
            tc.tile_pool(name="work", bufs=2) as wkp,
            tc.tile_pool(name="psProj", bufs=2, space="PSUM") as psProj,
            tc.tile_pool(name="psSim", bufs=2, space="PSUM") as psSim,
            tc.tile_pool(name="psPV", bufs=2, space="PSUM") as psPV,
        ):
            # ---------------- init: weights, biases, X^T ---------------------
            wq_b = pp.tile([128, NCH * FPC], BF16)
            wk_b = pp.tile([128, NCH * FPC], BF16)
            wv_b = pp.tile([128, NCH * VW], BF16)
            bq_sb = pp.tile([128, 1], F32)
            bk_sb = pp.tile([128, 1], F32)

            # X^T per (batch, chunk): fine-grained tiles so projection
            # dependencies resolve per DMA, batch 0 first.
            xtc = [
                [pp.tile([128, S], BF16, name=f"xt_{b}_{c}") for c in range(NCH)]
                for b in range(B)
            ]

            # Small bias DMAs on the SWDGE (gpsimd) queue.  Each sync/HWDGE
            # dma_start costs ~650ns of ISSUE time regardless of size, so the
            # head-critical transfers use the fewest possible DMAs: one per
            # weight, one per (batch, chunk) for X^T — ordered wk, wq, batch-0
            # X^T (gates the first exp), wv, batch-1 X^T.
            nc.gpsimd.dma_start(bk_sb[:], bkc[:, :])
            nc.gpsimd.dma_start(bq_sb[:], bqc[:, :])
            nc.sync.dma_start(wk_b[:], wk[:, :])
            nc.sync.dma_start(wq_b[:], wq[:, :])
            nc.sync.dma_start(wv_b[:], wvp[:, :])
            for c in range(NCH):
                nc.sync.dma_start(
                    xtc[0][c][:, 0:2*TT], xt[c * 128 : (c + 1) * 128, 0:2*TT]
                )
            for c in range(NCH):
                nc.sync.dma_start(
                    xtc[0][c][:, 2*TT:S], xt[c * 128 : (c + 1) * 128, 2*TT:S]
                )
            for c in range(NCH):
                nc.sync.dma_start(xtc[1][c][:], xt[c * 128 : (c + 1) * 128, S : 2 * S])

            # ---------------- persistent activations ------------------------
            qt_sb = [
                [pp.tile([128, QT], BF16, name=f"qt_{b}_{u}") for u in range(NU)]
                for b in range(B)
            ]
            kt_sb = [
                [pp.tile([128, TT], BF16, name=f"kt_{b}_{t}") for t in range(NTPB)]
                for b in range(B)
            ]
            vp_sb = [
                [pp.tile([128, VW], BF16, name=f"vp_{b}_{j}") for j in range(S // 128)]
                for b in range(B)
            ]

            def proj_tile(b, t, w_b, bias_sb, dst):
                # Generator: two ~4-matmul granules, so deferred projections
                # trace in slack-sized pieces between attention iterations.
                tsl = slice(t * TT, (t + 1) * TT)
                ps = psProj.tile(
                    [128, TT], F32, name=f"pj_{b}_{t}_{dst.tensor.name}", tag="proj"
                )
                for c in range(NCH):
                    nc.tensor.matmul(
                        ps[:], w_b[:, c * FPC : (c + 1) * FPC], xtc[b][c][:, tsl],
                        start=(c == 0), stop=(c == NCH - 1),
                    )
                    if c in (1, 3, 5):
                        yield
                # PSUM->SBUF cast with the bias folded in (per-partition
                # scalar column) — runs on the DVE.
                nc.vector.tensor_scalar_add(dst[:], ps[:], bias_sb[:])
                yield

            def proj_vtile(b, j):
                # No bias matmul: softmax rows sum to 1, so bV is added on the
                # host; the denominator ones-columns are memset after the copy
                # (the wvp columns 64/129 are zero, so the PSUM there is 0).
                psv = psProj.tile([128, VW], F32, name=f"pv_{b}_{j}", tag="proj")
                for c in range(NCH):
                    nc.tensor.matmul(
                        psv[:], xtc[b][c][:, j * 128 : (j + 1) * 128],
                        wv_b[:, c * VW : (c + 1) * VW],
                        start=(c == 0), stop=(c == NCH - 1),
                    )
                    if c in (1, 3, 5):
                        yield
                nc.vector.tensor_copy(vp_sb[b][j][:], psv[:])
                nc.vector.memset(vp_sb[b][j][:, HS : HS + 1], 1.0)
                nc.vector.memset(vp_sb[b][j][:, VW - 1 : VW], 1.0)
                yield

            def chain(gens):
                for g in gens:
                    yield from g

            class StepQ:
                """Deferred-projection step queue: pull(target) traces steps
                until `target` have been traced (deadline-forced); drain_all
                flushes the remainder."""

                def __init__(self, gen):
                    self.it, self.n, self.done = gen, 0, False

                def pull(self, target):
                    while not self.done and self.n < target:
                        try:
                            next(self.it)
                            self.n += 1
                        except StopIteration:
                            self.done = True

                def drain_all(self):
                    self.pull(1 << 30)

            def proj_prefix(b):
                # Traced directly (not as steps): all of K plus Q of unit 0 —
                # the gate for the batch's first exp.  Q right after K(t0): Q
                # only needs the first X^T half, and the proj pool's 2-slot
                # FIFO would otherwise park it behind K(t2/t3)'s DMA wait.
                order = [(wk_b, bk_sb, kt_sb[b][0]), (wq_b, bq_sb, qt_sb[b][0])] + [
                    (wk_b, bk_sb, kt_sb[b][t]) for t in range(1, NTPB)
                ]
                for i, (w_b, bias_sb, dst) in enumerate(order):
                    t = 0 if i < 2 else i - 1
                    StepQ(proj_tile(b, t, w_b, bias_sb, dst)).drain_all()

            # Output staging: units land in persistent SBUF buffers; one big
            # 8KB-row DMA per (batch, head) at batch end (batch 0's overlaps
            # batch-1 attention, batch 1's is a short tail).
            obuf = [
                [pp.tile([HS + 1, S], F32, name=f"ob_{b}_{h}") for h in range(2)]
                for b in range(B)
            ]

            def attn_unit(b, u, sq=None, need=None):
                # sq/need: deferred-projection step queue and its cumulative
                # trace deadline per kt (vp[kt] must be traced before PV(kt)).
                pvp = [
                    psPV.tile([HS + 1, QT], F32, name=f"pvp_{b}_{u}_{h}", tag="pv")
                    for h in range(2)
                ]
                if sq and need:
                    sq.pull(need(0))
                for kt in range(NKT):
                    sim = psSim.tile([128, 2 * QT], F32, name=f"sim_{b}_{u}_{kt}", tag="sim")
                    # Both heads' sims in one tile: the two K=64 matmuls hit
                    # disjoint PE row groups and disjoint PSUM banks, and
                    # become ready together -> concurrent streaming.
                    for h in range(2):
                        hp = h * HS
                        nc.tensor.matmul(
                            sim[:, h * QT : (h + 1) * QT],
                            kt_sb[b][kt // 4][hp : hp + HS, (kt % 4) * KT : (kt % 4 + 1) * KT],
                            qt_sb[b][u][hp : hp + HS, :],
                            start=True, stop=True,
                            tile_position=(hp, 0),
                        )
                    pt = wkp.tile([128, 2 * QT], BF16, name=f"pt_{b}_{u}_{kt}", tag="pt", bufs=6)
                    nc.scalar.activation(
                        pt[:], sim[:], mybir.ActivationFunctionType.Exp, scale=1.0 / np.sqrt(HS)
                    )
                    for h in range(2):
                        nc.tensor.matmul(
                            pvp[h][:],
                            vp_sb[b][kt][:, h * (HS + 1) : (h + 1) * (HS + 1)],
                            pt[:, h * QT : (h + 1) * QT],
                            start=(kt == 0), stop=(kt == NKT - 1),
                        )
                    if sq and need and kt + 1 < NKT:
                        sq.pull(need(kt + 1))
                if sq:
                    sq.drain_all()
                for h in range(2):
                    nc.vector.tensor_copy(obuf[b][h][:, u * QT : (u + 1) * QT], pvp[h][:])
                if u % 2 == 1:
                    lo, hi = (u - 1) * QT, (u + 1) * QT
                    for h in range(2):
                        nc.sync.dma_start(
                            out[h * (HS + 1) : (h + 1) * (HS + 1), b * S + lo : b * S + hi],
                            obuf[b][h][:, lo:hi],
                        )

            # Emission order = scheduler priority; producers always trace
            # before consumers, but deferred projections trace in slack-sized
            # granules between attention iterations so the exp-paced stream
            # owns the priority and projections fill Tensor-engine gaps.
            def qgen(b, u):
                return proj_tile(b, u, wq_b, bq_sb, qt_sb[b][u])

            def vq_queue(b):
                # V' chunks (4 steps each) with the next unit's Q chain
                # embedded mid-queue so the Q->cast->sim chain never lands on
                # the unit boundary (a late PV is absorbed by the pt buffers;
                # a late sim stalls the exp stream directly).
                gens = [proj_vtile(b, j) for j in range(12)] + [qgen(b, 1)] + [
                    proj_vtile(b, j) for j in range(12, 16)
                ]
                return StepQ(chain(gens))

            def vq_need(kt):
                # Deadline-exact: vp[kt] (4 steps each) traced just before its
                # PV; the embedded Q chain (+4) pulled from kt 11 on.
                return 4 * (kt + 1) + (4 if kt >= 11 else 0)

            proj_prefix(0)
            attn_unit(0, 0, vq_queue(0), need=vq_need)
            attn_unit(
                0, 1,
                StepQ(chain([qgen(0, 2), qgen(0, 3)])),
                need=lambda kt: min(8, (kt + 3) // 2),
            )
            attn_unit(
                0, 2,
                StepQ(chain([proj_tile(1, t, wk_b, bk_sb, kt_sb[1][t]) for t in range(NTPB)])),
                need=lambda kt: kt + 1,
            )
            attn_unit(
                0, 3,
                StepQ(qgen(1, 0)),
                need=lambda kt: min(4, (kt + 1) // 3),
            )
            attn_unit(1, 0, vq_queue(1), need=vq_need)
            attn_unit(
                1, 1,
                StepQ(chain([qgen(1, 2), qgen(1, 3)])),
                need=lambda kt: min(8, (kt + 3) // 2),
            )
            attn_unit(1, 2)
            attn_unit(1, 3)

    nc.compile()
    return nc


def get_nc():
    if "nc" not in _NC_CACHE:
        _NC_CACHE["nc"] = build_nc()
    return _NC_CACHE["nc"]


def make_in_maps(seq_input, WQ, bQ, WK, bK, WV, bV):
    x = np.asarray(seq_input, dtype=np.float32).reshape(NTOK, D)
    xt = np.ascontiguousarray(x.T).astype(ml_dtypes.bfloat16)

    def sbuf_layout(w, width):
        # [D, width] -> [128, NCH*width] bf16: chunk c of D-rows lands at
        # columns [c*width, (c+1)*width) — the exact SBUF image the kernel
        # expects, pre-cast so no on-chip conversion is needed.
        return np.ascontiguousarray(
            w.reshape(NCH, 128, width).transpose(1, 0, 2).reshape(128, NCH * width)
        ).astype(ml_dtypes.bfloat16)

    in_maps = []
    for c in range(NCORES):
        lo, hi = c * FPC, (c + 1) * FPC
        wvp = np.zeros((D, VW), dtype=np.float32)
        wvp[:, 0:HS] = WV[:, lo : lo + HS]
        wvp[:, HS + 1 : 2 * HS + 1] = WV[:, lo + HS : hi]
        in_maps.append(
            {
                "xt": xt,
                "wq": sbuf_layout(np.ascontiguousarray(WQ[:, lo:hi]), FPC),
                "wk": sbuf_layout(np.ascontiguousarray(WK[:, lo:hi]), FPC),
                "wvp": sbuf_layout(wvp, VW),
                "bqc": np.ascontiguousarray(bQ[lo:hi]).reshape(FPC, 1),
                "bkc": np.ascontiguousarray(bK[lo:hi]).reshape(FPC, 1),
            }
        )
    return in_maps


def run(in_maps, trace=False):
    nc = get_nc()
    return bass_utils.run_bass_kernel_spmd(nc, in_maps, core_ids=list(range(NCORES)), trace=trace)


def kernel(seq_input, WQ, bQ, WK, bK, WV, bV):
    in_maps = make_in_maps(
        np.asarray(seq_input, np.float32),
        np.asarray(WQ, np.float32), np.asarray(bQ, np.float32),
        np.asarray(WK, np.float32), np.asarray(bK, np.float32),
        np.asarray(WV, np.float32), np.asarray(bV, np.float32),
    )
    res = run(in_maps)
    bV_np = np.asarray(bV, np.float32)
    parts = []
    for c in range(NCORES):
        o = res.results[c]["out"]  # [130, 4096] feature-major, unnormalized
        for h in range(2):
            lo = c * FPC + h * HS
            num = o[h * (HS + 1) : h * (HS + 1) + HS, :]      # [64, 4096]
            den = o[h * (HS + 1) + HS, :]                     # [4096]
            # softmax rows sum to 1, so the V bias is added after the fact
            parts.append((num / den).T + bV_np[lo : lo + HS])  # [4096, 64]
    full = np.concatenate(parts, axis=1)  # [4096, 1024]
    return full.reshape(B, S, H * HS)
